# revision 1
# baseline (speedup 1.0000x reference)
"""Trainium2 Bass kernel for nn_DCTFeatureModel.

Math: the reference pipeline (3D DCT-II over [time-in-bin, H, W], mean over
DCT bins, full-receptive-field Conv3d, bias, LeakyReLU) is linear up to the
LeakyReLU, so everything folds into a single small matmul:

    feat[b,s,o] = LeakyReLU( sum_{c,t,i,j} x[b,s,c,t,i,j] * Weff[s,o,t,i,j]
                             + bias[s,o] )
    Weff[s,o,t,i,j] = (1/8) * sum_{f,p,q} Ct[f,t] Cs[p,i] Cs[q,j] W[s,o,f,p,q]

Weff is tiny (2*64*2048 floats) and computed on host. The device kernel is
memory-bound: stream x (134 MB full / 16.8 MB per core), reduce over the 8
DCT bins (c), then a [128b x 2048k] @ [2048k x 64o] matmul per subwindow.

Device dataflow (per core, fp32 exact): the host lays each core's x shard
out as contiguous [kin=128, chin*b = 1024] blocks per (s, c, g) so every
DMA unit is a fully contiguous 512 KB load arriving with the contraction
index already on partitions, and all 8 c-blocks of a (s, g) group land
within a ~10 us window. The c-reduction is a binary tree per group split
across DVE and GPSIMD (each engine's chain kept engine-local, one cross
join at the root); each reduced tile's 128-col slices are directly the
transposed matmul operands (no PE transposes, no PSUM->SBUF copies).
PE does 32 accumulating fp32 matmuls + 2 rank-1 bias matmuls;
LeakyReLU = max(v, 0.02v) on DVE.

Sharding: pure data-parallel over batch, 1024/8 = 128 rows per core.
"""

from contextlib import ExitStack

import numpy as np

import concourse.bacc as bacc
import concourse.tile as tile
from concourse import mybir
from concourse.bass_utils import run_bass_kernel_spmd

# Problem shapes (hardcoded per contract)
B = 1024
NCORES = 8
BS = B // NCORES          # 128 batch rows per core
NSW = 2                   # subwindows
NBINS = 8                 # DCT bins (mean-reduced)
NDCT = 32                 # time points per bin
HW = 8
NF = 64                   # conv output filters per subwindow
K = NDCT * HW * HW        # 2048 contraction elements per (s, c)
P = 128                   # partitions
NCHUNK = K // P           # 16 k-chunks of 128
NG = 2                    # chunk-groups per s
CPG = NCHUNK // NG        # 8 chunks per group
GW = CPG * P              # 1024 columns per group tile
OUT_F = NSW * NF          # 128 output features
SLOPE = 0.02

F32 = mybir.dt.float32

_cached = None
last_results = None


def _dct2(N):
    n = np.arange(N, dtype=np.float64)
    k = np.arange(N, dtype=np.float64)
    return 2.0 * np.cos(np.pi * (2.0 * n[None, :] + 1.0) * k[:, None] / (2.0 * N))


def _kernel_body(tc, x, w, bias, out):
    """x: [NSW*NBINS*NG, 128, GW] (s,c,g blocks, each [kin, chin*b], contiguous)
    w: [P, NSW*NCHUNK*NF]; bias: [1, OUT_F]; out: [BS, OUT_F]"""
    nc = tc.nc
    with ExitStack() as ctx:
        const_pool = ctx.enter_context(tc.tile_pool(name="const", bufs=1))
        xpool = ctx.enter_context(tc.tile_pool(name="xp", bufs=14))
        tpool = ctx.enter_context(tc.tile_pool(name="tp", bufs=8))
        zpool = ctx.enter_context(tc.tile_pool(name="zp", bufs=3))
        opool = ctx.enter_context(tc.tile_pool(name="op", bufs=1))
        pft_pool = ctx.enter_context(tc.tile_pool(name="pft", bufs=1, space="PSUM"))

        # consts dispatched off the sync engine so the x stream starts at once
        w_sb = const_pool.tile([P, NSW * NCHUNK * NF], F32)
        nc.scalar.dma_start(out=w_sb, in_=w)
        bias_sb = const_pool.tile([1, OUT_F], F32)
        nc.scalar.dma_start(out=bias_sb, in_=bias)
        ones = const_pool.tile([1, P], F32)
        nc.gpsimd.memset(ones, 1.0)

        out_sb = opool.tile([BS, OUT_F], F32)
        psum_feat = [
            pft_pool.tile([P, NF], F32, tag=f"feat{s}", name=f"psum_feat{s}")
            for s in range(NSW)
        ]

        for s in range(NSW):
            for g in range(NG):
                # --- load the 8 c-slices of this (s, g): contiguous [128, 1024] ---
                tiles = []
                for c in range(NBINS):
                    t = xpool.tile([P, GW], F32, tag="x", name=f"x_{s}_{g}_{c}")
                    nc.sync.dma_start(out=t, in_=x[(s * NBINS + c) * NG + g])
                    tiles.append(t)
                # --- binary tree c-reduction on DVE + GPSIMD ---
                # lvl0: (01)->DVE (23)->GP (45)->GP (67)->DVE   (67 gates the tail)
                l0 = []
                engs = [nc.vector, nc.gpsimd, nc.gpsimd, nc.vector]
                for j in range(4):
                    u = tpool.tile([P, GW], F32, tag="tree", name=f"t_{s}_{g}_{j}")
                    engs[j].tensor_add(out=u, in0=tiles[2 * j], in1=tiles[2 * j + 1])
                    l0.append(u)
                v0 = tpool.tile([P, GW], F32, tag="tree", name=f"v0_{s}_{g}")
                nc.gpsimd.tensor_add(out=v0, in0=l0[0], in1=l0[1])
                v1 = tpool.tile([P, GW], F32, tag="tree", name=f"v1_{s}_{g}")
                nc.vector.tensor_add(out=v1, in0=l0[2], in1=l0[3])
                z = zpool.tile([P, GW], F32, tag="z", name=f"z_{s}_{g}")
                nc.vector.tensor_add(out=z, in0=v0, in1=v1)

                # --- PE: each 128-col slice of z is a ready lhsT chunk ---
                for j in range(CPG):
                    ch = g * CPG + j
                    nc.tensor.matmul(
                        psum_feat[s],
                        lhsT=z[:, j * P:(j + 1) * P],
                        rhs=w_sb[:, (s * NCHUNK + ch) * NF:(s * NCHUNK + ch + 1) * NF],
                        start=(ch == 0),
                        stop=False,
                    )

        for s in range(NSW):
            # bias via rank-1 matmul: ones[1, b].T @ bias[1, o]
            nc.tensor.matmul(
                psum_feat[s],
                lhsT=ones,
                rhs=bias_sb[:, s * NF:(s + 1) * NF],
                start=False,
                stop=True,
            )
            # LeakyReLU(v) = max(v, slope*v)  (slope < 1)
            tmp = tpool.tile([P, NF], F32, tag="lrelu", name=f"lr_{s}")
            nc.vector.tensor_scalar_mul(tmp, psum_feat[s], SLOPE)
            nc.vector.tensor_max(
                out=out_sb[:, s * NF:(s + 1) * NF], in0=psum_feat[s], in1=tmp
            )

        nc.sync.dma_start(out=out, in_=out_sb)


def _build():
    global _cached
    if _cached is not None:
        return _cached
    nc = bacc.Bacc(
        "TRN2",
        target_bir_lowering=False,
        debug=False,
        enable_asserts=False,
        num_devices=NCORES,
    )
    x_ap = nc.dram_tensor(
        "x", [NSW * NBINS * NG, P, GW], F32, kind="ExternalInput"
    ).ap()
    w_ap = nc.dram_tensor("w", [P, NSW * NCHUNK * NF], F32, kind="ExternalInput").ap()
    b_ap = nc.dram_tensor("bias", [1, OUT_F], F32, kind="ExternalInput").ap()
    out_ap = nc.dram_tensor("out", [BS, OUT_F], F32, kind="ExternalOutput").ap()
    with tile.TileContext(nc, trace_sim=False) as tc:
        _kernel_body(tc, x_ap, w_ap, b_ap, out_ap)
    nc.compile()
    _cached = nc
    return nc


def kernel(x, W, b):
    global last_results
    assert x.shape == (B, 1, NSW * NBINS * NDCT, HW, HW), x.shape
    nc = _build()

    # Host-side folding of the DCT matrices into the conv weights (tiny).
    Ct = _dct2(NDCT)                       # [f, t]
    Cs = _dct2(HW)                         # [p, i]
    Weff = np.einsum(
        "ft,pi,qj,sofpq->sotij", Ct, Cs, Cs, W.astype(np.float64), optimize=True
    ) / float(NBINS)
    Weff_k = Weff.reshape(NSW, NF, K)      # [s, o, k]
    # device layout: w[p, s*NCHUNK*NF + ch*NF + o] = Weff_k[s, o, ch*128 + p]
    w_dev = np.ascontiguousarray(
        Weff_k.reshape(NSW, NF, NCHUNK, P).transpose(3, 0, 2, 1).reshape(P, NSW * NCHUNK * NF)
    ).astype(np.float32)
    bias_dev = np.ascontiguousarray(b.reshape(1, OUT_F)).astype(np.float32)

    x2 = x.reshape(B, NSW, NBINS, NG, CPG, P)  # (b, s, c, g, chin, kin)
    in_maps = []
    for i in range(NCORES):
        xs = x2[i * BS:(i + 1) * BS]
        # -> [s, c, g, kin, chin, b]: one contiguous [128, 1024] block per (s,c,g)
        xt = np.ascontiguousarray(xs.transpose(1, 2, 3, 5, 4, 0)).reshape(
            NSW * NBINS * NG, P, GW
        )
        in_maps.append({"x": xt, "w": w_dev, "bias": bias_dev})
    res = run_bass_kernel_spmd(nc, in_maps, core_ids=list(range(NCORES)))
    last_results = res
    return np.concatenate([r["out"] for r in res.results], axis=0)



# revision 2
# speedup vs baseline: 1.3758x; 1.3758x over previous
"""Trainium2 Bass kernel for nn_DCTFeatureModel.

Math: the reference pipeline (3D DCT-II over [time-in-bin, H, W], mean over
DCT bins, full-receptive-field Conv3d, bias, LeakyReLU) is linear up to the
LeakyReLU, so everything folds into a single small matmul:

    feat[b,s,o] = LeakyReLU( sum_{c,t,i,j} x[b,s,c,t,i,j] * Weff[s,o,t,i,j]
                             + bias[s,o] )
    Weff[s,o,t,i,j] = (1/8) * sum_{f,p,q} Ct[f,t] Cs[p,i] Cs[q,j] W[s,o,f,p,q]

Weff is tiny (2*64*2048 floats) and computed on host. The device kernel is
memory-bound: stream x, reduce over the 8 DCT bins (c), then a
[128b x 2048k] @ [2048k x 64o] matmul per subwindow.

v2 changes vs v1 (76.5 us):
  * x, Weff, bias shipped as fp16 — halves HBM traffic (16.8 -> 8.4 MB per
    core). Tolerance is 2e-2; fp16 keeps the error ~1e-3.
  * One 2 MB DMA per (s, g) group instead of 8 x 512 KB: descriptor rows grow
    4 KB -> 16 KB and dma_start issue cost on the Sync sequencer (628 ns per
    DIRECT2D) drops 8x. All 4 loads are issued up-front (xpool bufs=4).
  * c-reduction tree (7 adds per group, split DVE/GPSIMD) now runs at the
    2x 16-bit element rate; matmuls run at the fp16 PE rate.

Sharding: pure data-parallel over batch, 1024/8 = 128 rows per core.
"""

from contextlib import ExitStack

import numpy as np

import concourse.bacc as bacc
import concourse.tile as tile
from concourse import mybir
from concourse.bass_utils import run_bass_kernel_spmd

# Problem shapes (hardcoded per contract)
B = 1024
NCORES = 8
BS = B // NCORES          # 128 batch rows per core
NSW = 2                   # subwindows
NBINS = 8                 # DCT bins (mean-reduced)
NDCT = 32                 # time points per bin
HW = 8
NF = 64                   # conv output filters per subwindow
K = NDCT * HW * HW        # 2048 contraction elements per (s, c)
P = 128                   # partitions
NCHUNK = K // P           # 16 k-chunks of 128
NG = 2                    # chunk-groups per s
CPG = NCHUNK // NG        # 8 chunks per group
GW = CPG * P              # 1024 columns per group tile (chin, b)
OUT_F = NSW * NF          # 128 output features
SLOPE = 0.02

F32 = mybir.dt.float32
F16 = mybir.dt.float16

_cached = None
last_results = None


def _dct2(N):
    n = np.arange(N, dtype=np.float64)
    k = np.arange(N, dtype=np.float64)
    return 2.0 * np.cos(np.pi * (2.0 * n[None, :] + 1.0) * k[:, None] / (2.0 * N))


def _kernel_body(tc, x, w, bias, out):
    """x: [NSW*NG, 128, NBINS*GW] fp16 — one (s,g) mega-block, cols (c, chin, b)
    w: [P, NSW*NCHUNK*NF] fp16; bias: [1, OUT_F] fp16; out: [BS, OUT_F] fp32"""
    nc = tc.nc
    with ExitStack() as ctx:
        const_pool = ctx.enter_context(tc.tile_pool(name="const", bufs=1))
        xpool = ctx.enter_context(tc.tile_pool(name="xp", bufs=4))
        tpool = ctx.enter_context(tc.tile_pool(name="tp", bufs=8))
        zpool = ctx.enter_context(tc.tile_pool(name="zp", bufs=3))
        opool = ctx.enter_context(tc.tile_pool(name="op", bufs=1))
        pft_pool = ctx.enter_context(tc.tile_pool(name="pft", bufs=1, space="PSUM"))

        # consts dispatched off the scalar engine so the x stream starts at once
        w_sb = const_pool.tile([P, NSW * NCHUNK * NF], F16)
        nc.scalar.dma_start(out=w_sb, in_=w)
        bias_sb = const_pool.tile([1, OUT_F], F16)
        nc.scalar.dma_start(out=bias_sb, in_=bias)
        ones = const_pool.tile([1, P], F16)
        nc.gpsimd.memset(ones, 1.0)

        out_sb = opool.tile([BS, OUT_F], F32)
        psum_feat = [
            pft_pool.tile([P, NF], F32, tag=f"feat{s}", name=f"psum_feat{s}")
            for s in range(NSW)
        ]

        # all 4 mega-loads issued up-front: queues saturate, zero WAR stalls
        xt = []
        for s in range(NSW):
            for g in range(NG):
                t = xpool.tile([P, NBINS * GW], F16, tag="x", name=f"x_{s}_{g}")
                nc.sync.dma_start(out=t, in_=x[s * NG + g])
                xt.append(t)

        for s in range(NSW):
            for g in range(NG):
                T = xt[s * NG + g]
                # --- binary tree c-reduction on DVE + GPSIMD ---
                l0 = []
                engs = [nc.vector, nc.gpsimd, nc.gpsimd, nc.vector]
                for j in range(4):
                    u = tpool.tile([P, GW], F16, tag="tree", name=f"t_{s}_{g}_{j}")
                    engs[j].tensor_add(
                        out=u,
                        in0=T[:, (2 * j) * GW:(2 * j + 1) * GW],
                        in1=T[:, (2 * j + 1) * GW:(2 * j + 2) * GW],
                    )
                    l0.append(u)
                v0 = tpool.tile([P, GW], F16, tag="tree", name=f"v0_{s}_{g}")
                nc.gpsimd.tensor_add(out=v0, in0=l0[0], in1=l0[1])
                v1 = tpool.tile([P, GW], F16, tag="tree", name=f"v1_{s}_{g}")
                nc.vector.tensor_add(out=v1, in0=l0[2], in1=l0[3])
                z = zpool.tile([P, GW], F16, tag="z", name=f"z_{s}_{g}")
                nc.vector.tensor_add(out=z, in0=v0, in1=v1)

                # --- PE: each 128-col slice of z is a ready lhsT chunk ---
                for j in range(CPG):
                    ch = g * CPG + j
                    nc.tensor.matmul(
                        psum_feat[s],
                        lhsT=z[:, j * P:(j + 1) * P],
                        rhs=w_sb[:, (s * NCHUNK + ch) * NF:(s * NCHUNK + ch + 1) * NF],
                        start=(ch == 0),
                        stop=False,
                    )

            # bias via rank-1 matmul: ones[1, b].T @ bias[1, o]
            nc.tensor.matmul(
                psum_feat[s],
                lhsT=ones,
                rhs=bias_sb[:, s * NF:(s + 1) * NF],
                start=False,
                stop=True,
            )
            # LeakyReLU(v) = max(v, slope*v)  (slope < 1)
            tmp = tpool.tile([P, NF], F32, tag="lrelu", name=f"lr_{s}")
            nc.vector.tensor_scalar_mul(tmp, psum_feat[s], SLOPE)
            nc.vector.tensor_max(
                out=out_sb[:, s * NF:(s + 1) * NF], in0=psum_feat[s], in1=tmp
            )

        nc.sync.dma_start(out=out, in_=out_sb)


def _build():
    global _cached
    if _cached is not None:
        return _cached
    nc = bacc.Bacc(
        "TRN2",
        target_bir_lowering=False,
        debug=False,
        enable_asserts=False,
        num_devices=NCORES,
    )
    x_ap = nc.dram_tensor(
        "x", [NSW * NG, P, NBINS * GW], F16, kind="ExternalInput"
    ).ap()
    w_ap = nc.dram_tensor("w", [P, NSW * NCHUNK * NF], F16, kind="ExternalInput").ap()
    b_ap = nc.dram_tensor("bias", [1, OUT_F], F16, kind="ExternalInput").ap()
    out_ap = nc.dram_tensor("out", [BS, OUT_F], F32, kind="ExternalOutput").ap()
    with tile.TileContext(nc, trace_sim=False) as tc:
        _kernel_body(tc, x_ap, w_ap, b_ap, out_ap)
    nc.compile()
    _cached = nc
    return nc


def kernel(x, W, b):
    global last_results
    assert x.shape == (B, 1, NSW * NBINS * NDCT, HW, HW), x.shape
    nc = _build()

    # Host-side folding of the DCT matrices into the conv weights (tiny).
    Ct = _dct2(NDCT)                       # [f, t]
    Cs = _dct2(HW)                         # [p, i]
    Weff = np.einsum(
        "ft,pi,qj,sofpq->sotij", Ct, Cs, Cs, W.astype(np.float64), optimize=True
    ) / float(NBINS)
    Weff_k = Weff.reshape(NSW, NF, K)      # [s, o, k]
    # device layout: w[p, s*NCHUNK*NF + ch*NF + o] = Weff_k[s, o, ch*128 + p]
    w_dev = np.ascontiguousarray(
        Weff_k.reshape(NSW, NF, NCHUNK, P).transpose(3, 0, 2, 1).reshape(P, NSW * NCHUNK * NF)
    ).astype(np.float16)
    bias_dev = np.ascontiguousarray(b.reshape(1, OUT_F)).astype(np.float16)

    x2 = x.reshape(B, NSW, NBINS, NG, CPG, P)  # (b, s, c, g, chin, kin)
    in_maps = []
    for i in range(NCORES):
        xs = x2[i * BS:(i + 1) * BS]
        # -> [s, g, kin, c, chin, b]: one contiguous [128, 8192] block per (s,g)
        xt = np.ascontiguousarray(
            xs.transpose(1, 3, 5, 2, 4, 0).astype(np.float16)
        ).reshape(NSW * NG, P, NBINS * GW)
        in_maps.append({"x": xt, "w": w_dev, "bias": bias_dev})
    res = run_bass_kernel_spmd(nc, in_maps, core_ids=list(range(NCORES)))
    last_results = res
    return np.concatenate([r["out"] for r in res.results], axis=0)


# revision 4
# speedup vs baseline: 1.8063x; 1.3129x over previous
"""Trainium2 Bass kernel for nn_DCTFeatureModel.

Math: the reference pipeline (3D DCT-II over [time-in-bin, H, W], mean over
DCT bins, full-receptive-field Conv3d, bias, LeakyReLU) is linear up to the
LeakyReLU, so everything folds into a single small matmul:

    feat[b,s,o] = LeakyReLU( sum_{c,t,i,j} x[b,s,c,t,i,j] * Weff[s,o,t,i,j]
                             + bias[s,o] )
    Weff[s,o,t,i,j] = (1/8) * sum_{f,p,q} Ct[f,t] Cs[p,i] Cs[q,j] W[s,o,f,p,q]

Weff is tiny and computed on host. The device kernel is memory-bound:
stream x (fp16, 8.4 MB per core), reduce over the 8 DCT bins (c), then a
[128b x 2048k] @ [2048k x 64o] matmul per subwindow.

v3 (vs v2, 55 us): the v2 trace showed GPSIMD fp16 adds run at only
~50 G elem/s (2.5 us each, 3 serial per group = the critical path) while
DVE fp16 adds take 0.6 us and fp16 matmuls 53 ns. So: c-pairs are summed
on DVE only (lvl0, 4 adds per group), and the remaining 4-way reduction
rides the PE's PSUM accumulation (4 partial-sum matmuls per k-chunk).
x ships as 16 pair-blocks [128, 2048] (512 KB DMAs, 4 KB rows) for finer
load->reduce pipelining; output halves are DMA'd from the Vector queue
right after each subwindow's LeakyReLU.

Sharding: pure data-parallel over batch, 1024/8 = 128 rows per core.
"""

from contextlib import ExitStack

import numpy as np

import concourse.bacc as bacc
import concourse.tile as tile
from concourse import mybir
from concourse.bass_utils import run_bass_kernel_spmd

# Problem shapes (hardcoded per contract)
B = 1024
NCORES = 8
BS = B // NCORES          # 128 batch rows per core
NSW = 2                   # subwindows
NBINS = 8                 # DCT bins (mean-reduced)
NPAIR = NBINS // 2        # 4 c-pairs per group
NDCT = 32                 # time points per bin
HW = 8
NF = 64                   # conv output filters per subwindow
K = NDCT * HW * HW        # 2048 contraction elements per (s, c)
P = 128                   # partitions
NCHUNK = K // P           # 16 k-chunks of 128
NG = 2                    # chunk-groups per s
CPG = NCHUNK // NG        # 8 chunks per group
GW = CPG * P              # 1024 columns per (chin, b) block
OUT_F = NSW * NF          # 128 output features
SLOPE = 0.02

F32 = mybir.dt.float32
F16 = mybir.dt.float16

_cached = None
last_results = None


def _dct2(N):
    n = np.arange(N, dtype=np.float64)
    k = np.arange(N, dtype=np.float64)
    return 2.0 * np.cos(np.pi * (2.0 * n[None, :] + 1.0) * k[:, None] / (2.0 * N))


def _kernel_body(tc, x, w, bias, out):
    """x: [NSW*NG*NPAIR, 128, 2*GW] fp16 — pair-blocks, cols (c_parity, chin, b)
    w: [P, NSW*NCHUNK*NF] fp16; bias: [1, OUT_F] fp16; out: [BS, OUT_F] fp32"""
    nc = tc.nc
    with ExitStack() as ctx:
        const_pool = ctx.enter_context(tc.tile_pool(name="const", bufs=1))
        xpool = ctx.enter_context(tc.tile_pool(name="xp", bufs=16))
        upool = ctx.enter_context(tc.tile_pool(name="up", bufs=8))
        opool = ctx.enter_context(tc.tile_pool(name="op", bufs=1))
        pft_pool = ctx.enter_context(tc.tile_pool(name="pft", bufs=1, space="PSUM"))

        # consts dispatched off the scalar engine so the x stream starts at once
        w_sb = const_pool.tile([P, NSW * NCHUNK * NF], F16)
        nc.scalar.dma_start(out=w_sb, in_=w)
        bias_sb = const_pool.tile([1, OUT_F], F16)
        nc.scalar.dma_start(out=bias_sb, in_=bias)
        ones = const_pool.tile([1, P], F16)
        nc.gpsimd.memset(ones, 1.0)

        out_sb = opool.tile([BS, OUT_F], F32)
        psum_feat = [
            pft_pool.tile([P, NF], F32, tag=f"feat{s}", name=f"psum_feat{s}")
            for s in range(NSW)
        ]

        # all 16 pair-loads issued up-front, alternating sync/scalar queues
        xt = []
        for i in range(NSW * NG * NPAIR):
            t = xpool.tile([P, 2 * GW], F16, tag="x", name=f"x_{i}")
            eng = nc.sync if (i % 2 == 0) else nc.scalar
            eng.dma_start(out=t, in_=x[i])
            xt.append(t)

        for s in range(NSW):
            for g in range(NG):
                # lvl0 c-pair sums on DVE (0.6 us each); PE accumulates the
                # 4 partials into PSUM, so no lvl1/root adds are needed.
                for m in range(NPAIR):
                    pt = xt[(s * NG + g) * NPAIR + m]
                    u = upool.tile([P, GW], F16, tag="u", name=f"u_{s}_{g}_{m}")
                    nc.vector.tensor_add(out=u, in0=pt[:, :GW], in1=pt[:, GW:])
                    for j in range(CPG):
                        ch = g * CPG + j
                        nc.tensor.matmul(
                            psum_feat[s],
                            lhsT=u[:, j * P:(j + 1) * P],
                            rhs=w_sb[:, (s * NCHUNK + ch) * NF:(s * NCHUNK + ch + 1) * NF],
                            start=(g == 0 and m == 0 and j == 0),
                            stop=False,
                        )

            # bias via rank-1 matmul: ones[1, b].T @ bias[1, o]
            nc.tensor.matmul(
                psum_feat[s],
                lhsT=ones,
                rhs=bias_sb[:, s * NF:(s + 1) * NF],
                start=False,
                stop=True,
            )
            # LeakyReLU(v) = max(v, slope*v)  (slope < 1)
            tmp = upool.tile([P, NF], F32, tag="lrelu", name=f"lr_{s}")
            nc.vector.tensor_scalar_mul(tmp, psum_feat[s], SLOPE)
            nc.vector.tensor_max(
                out=out_sb[:, s * NF:(s + 1) * NF], in0=psum_feat[s], in1=tmp
            )
            # ship this subwindow's half right away (gpsimd queue is idle)
            nc.gpsimd.dma_start(
                out=out[:, s * NF:(s + 1) * NF], in_=out_sb[:, s * NF:(s + 1) * NF]
            )


def _build():
    global _cached
    if _cached is not None:
        return _cached
    nc = bacc.Bacc(
        "TRN2",
        target_bir_lowering=False,
        debug=False,
        enable_asserts=False,
        num_devices=NCORES,
    )
    x_ap = nc.dram_tensor(
        "x", [NSW * NG * NPAIR, P, 2 * GW], F16, kind="ExternalInput"
    ).ap()
    w_ap = nc.dram_tensor("w", [P, NSW * NCHUNK * NF], F16, kind="ExternalInput").ap()
    b_ap = nc.dram_tensor("bias", [1, OUT_F], F16, kind="ExternalInput").ap()
    out_ap = nc.dram_tensor("out", [BS, OUT_F], F32, kind="ExternalOutput").ap()
    with tile.TileContext(nc, trace_sim=False) as tc:
        _kernel_body(tc, x_ap, w_ap, b_ap, out_ap)
    nc.compile()
    _cached = nc
    return nc


def kernel(x, W, b):
    global last_results
    assert x.shape == (B, 1, NSW * NBINS * NDCT, HW, HW), x.shape
    nc = _build()

    # Host-side folding of the DCT matrices into the conv weights (tiny).
    Ct = _dct2(NDCT)                       # [f, t]
    Cs = _dct2(HW)                         # [p, i]
    Weff = np.einsum(
        "ft,pi,qj,sofpq->sotij", Ct, Cs, Cs, W.astype(np.float64), optimize=True
    ) / float(NBINS)
    Weff_k = Weff.reshape(NSW, NF, K)      # [s, o, k]
    # device layout: w[p, s*NCHUNK*NF + ch*NF + o] = Weff_k[s, o, ch*128 + p]
    w_dev = np.ascontiguousarray(
        Weff_k.reshape(NSW, NF, NCHUNK, P).transpose(3, 0, 2, 1).reshape(P, NSW * NCHUNK * NF)
    ).astype(np.float16)
    bias_dev = np.ascontiguousarray(b.reshape(1, OUT_F)).astype(np.float16)

    # (b, s, m, ce, g, chin, kin): c split into pair index m and parity ce
    x2 = x.reshape(B, NSW, NPAIR, 2, NG, CPG, P)
    in_maps = []
    for i in range(NCORES):
        xs = x2[i * BS:(i + 1) * BS]
        # -> [s, g, m, kin, ce, chin, b]: contiguous [128, 2048] per pair-block
        xt = np.ascontiguousarray(
            xs.transpose(1, 4, 2, 6, 3, 5, 0).astype(np.float16)
        ).reshape(NSW * NG * NPAIR, P, 2 * GW)
        in_maps.append({"x": xt, "w": w_dev, "bias": bias_dev})
    res = run_bass_kernel_spmd(nc, in_maps, core_ids=list(range(NCORES)))
    last_results = res
    return np.concatenate([r["out"] for r in res.results], axis=0)
